# revision 17
# baseline (speedup 1.0000x reference)
"""Trainium2 Bass kernel for MultiHeadMemAttn (mean-pooled-memory attention).

Full computation (per batch b):
    mem  = mean_pool(keyvalue, window=64, stride=64)          # [64, 512]
    hq   = query @ Wq.T ; hk = mem @ Wk.T ; hv = mem @ Wv.T   # heads=8, hd=64
    attn = softmax(hq @ hk.T / 8, over mem axis)
    out  = (attn @ hv) @ Wo.T

Sharding: data-parallel over batch across 8 cores (4 batches each),
weights replicated.  No collectives.

Device strategy (per core), v2 (bf16 dataflow):
  - ALL tensors bf16 on device (host casts inputs, upcasts the output):
    halves HBM traffic vs f32 and enables FWL (2x faster weight loads).
  - pooling as PE matmul against a host-built band matrix (32 s-chunks
    accumulated in PSUM per batch).
  - scores computed transposed (scoresT[j, i] = hkbd_pair.T @ hqT_pair),
    head pairs packed into one [128, 512] tile via block-diagonal hkbd.
  - softmax: E = exp(s/8) on ACT; denominators via a ones-matmul into a
    single [8, 512] PSUM tile; reciprocal as exp(-ln(den)) on ACT (Ln and
    Exp share one ACT table -> no table thrash, and this avoids the very
    slow DVE reciprocal); denominators broadcast via an [8->128] expand
    matmul; normalize on DVE.
  - output projection FUSED with the V-aggregation:
      hvo_h = hv_h @ Wo_h  (per head, [64, 512], done once per batch)
      out[i,:] = sum_h attn_h[:, i].T @ hvo_h
    which removes the separate uv matmul and the vecT copies entirely.
    attn tiles [j-packed, i] serve directly as matmul lhsT.
"""

import os
from contextlib import ExitStack

import numpy as np
import ml_dtypes

import concourse.bass as bass
import concourse.mybir as mybir
import concourse.tile as tile
from concourse.bass_utils import run_bass_kernel_spmd

F32 = mybir.dt.float32
BF16 = mybir.dt.bfloat16
F32R = mybir.dt.float32r

NCORES = 8
B = 4          # batches per core
QLEN = 1024
S = 4096       # kv sequence length
D = 512        # hidden
H = 8          # heads
HD = 64        # head dim
MEM = 64       # mem_len (pooled length)
DC = D // 128  # 4 chunks of the hidden dim
ICN = 2        # i-chunks of 512 per batch
IT = 4         # 128-row tiles per i-chunk
KT = 8         # kv DMA tiles per batch (4 s-chunks of 128 each)

EXPF = mybir.ActivationFunctionType.Exp
LNF = mybir.ActivationFunctionType.Ln

# ---------------------------------------------------------------------------
# Workaround: this walrus build only encodes ONE sem-wait per instruction
# ("Too many sync wait commands" in CoreV3GenImpl setupSyncWait), while
# Tile's sem-assignment freely attaches several.  Post-process the
# serialized BIR: move surplus waits onto injected same-engine NoOps placed
# immediately before the instruction (engine streams are in-order, so the
# NoOp chain stalls the engine exactly like multi-wait would).
import json as _json

_orig_to_json_bytes = bass.Bass.to_json_bytes


def _split_multi_waits(self, *args, **kwargs):
    raw = _orig_to_json_bytes(self, *args, **kwargs)
    d = _json.loads(raw)
    changed = False

    def fix_block(o):
        nonlocal changed
        if isinstance(o, dict):
            insts = o.get("instructions")
            if isinstance(insts, list):
                new = []
                for inst in insts:
                    si = inst.get("sync_info") if isinstance(inst, dict) else None
                    waits = (si or {}).get("on_wait") or []
                    if len(waits) > 1:
                        changed = True
                        for i, w in enumerate(waits[:-1]):
                            new.append(
                                {
                                    "name": f"{inst['name']}-sw{i}",
                                    "opcode": "NoOp",
                                    "engine": inst["engine"],
                                    "ins": [],
                                    "outs": [],
                                    "debug": inst.get("debug", 0),
                                    "sync_info": {
                                        "on_wait": [w],
                                        "on_update": [],
                                    },
                                }
                            )
                        si["on_wait"] = [waits[-1]]
                    new.append(inst)
                o["instructions"] = new
            for v in o.values():
                fix_block(v)
        elif isinstance(o, list):
            for v in o:
                fix_block(v)

    fix_block(d)
    if not changed:
        return raw
    return _json.dumps(d).encode()


bass.Bass.to_json_bytes = _split_multi_waits
# ---------------------------------------------------------------------------


def _build_nc() -> bass.Bass:
    nc = bass.Bass()
    q = nc.dram_tensor("queryT", [B, D, QLEN], BF16, kind="ExternalInput")
    kv = nc.dram_tensor("keyvalue", [B, S, D], BF16, kind="ExternalInput")
    wqT = nc.dram_tensor("wqT", [D, D], BF16, kind="ExternalInput")
    wkT = nc.dram_tensor("wkT", [D, D], BF16, kind="ExternalInput")
    wvT = nc.dram_tensor("wvT", [D, D], BF16, kind="ExternalInput")
    woT = nc.dram_tensor("woT", [D, D], BF16, kind="ExternalInput")
    poolD = nc.dram_tensor("poolD", [128, 126], BF16, kind="ExternalInput")
    ident = nc.dram_tensor("ident", [128, 128], F32R, kind="ExternalInput")
    ones8 = nc.dram_tensor("ones8", [128, 4, 8], BF16, kind="ExternalInput")
    expand8 = nc.dram_tensor("expand8", [8, 4, 128], F32R, kind="ExternalInput")
    out = nc.dram_tensor("out", [B, QLEN, D], BF16, kind="ExternalOutput")

    # DRAM views for partition-major DMA
    q_v = q.rearrange("b (dc p) (ic i) -> b ic p dc i", p=128, ic=ICN)
    kv_v = kv.rearrange("b (t c p) d -> b t p c d", t=KT, c=4, p=128)
    out_v = out.rearrange("b (ic it p) d -> b ic p it d", ic=ICN, it=IT, p=128)
    wq_v = wqT.rearrange("(dc p) o -> p dc o", p=128)
    wk_v = wkT.rearrange("(dc p) o -> p dc o", p=128)
    wv_v = wvT.rearrange("(dc p) o -> p dc o", p=128)
    wo_v = woT.rearrange("(dc p) o -> p dc o", p=128)

    with tile.TileContext(nc) as tc, ExitStack() as ctx:
        # SBUF pools
        singles = ctx.enter_context(tc.tile_pool(name="singles", bufs=1))
        kvp = ctx.enter_context(tc.tile_pool(name="kvp", bufs=8))
        qtp = ctx.enter_context(tc.tile_pool(name="qtp", bufs=4))
        hqp = ctx.enter_context(tc.tile_pool(name="hqp", bufs=3))
        memp = ctx.enter_context(tc.tile_pool(name="memp", bufs=2))
        ep = ctx.enter_context(tc.tile_pool(name="ep", bufs=9))
        attnp = ctx.enter_context(tc.tile_pool(name="attnp", bufs=5))
        rdp = ctx.enter_context(tc.tile_pool(name="rdp", bufs=3))
        outp = ctx.enter_context(tc.tile_pool(name="outp", bufs=2))
        # PSUM pools (8 banks total: 1 + 1 + 6)
        accp = ctx.enter_context(tc.tile_pool(name="accp", bufs=1, space="PSUM"))
        denp = ctx.enter_context(tc.tile_pool(name="denp", bufs=1, space="PSUM"))
        mmp = ctx.enter_context(tc.tile_pool(name="mmp", bufs=6, space="PSUM"))

        # one-time loads.  The sync ring carries what the first pool/hq
        # matmuls need (poolD, batch-0 kv, qT) in arrival order; all weight
        # matrices and the small attention consts go on the ACT ring, which
        # is otherwise idle until the first store (~30us in).  This gets
        # the first pool matmul started ~12us earlier than a single
        # serialized preamble queue.
        poolD_sb = singles.tile([128, 126], BF16)
        nc.sync.dma_start(out=poolD_sb, in_=poolD[:, :])
        kv0_tiles = {}
        qT0_pre = {}

        def _kv0(t):
            kvt = kvp.tile([128, 4, D], BF16, tag="kv", name=f"kv0_{t}")
            nc.sync.dma_start(out=kvt, in_=kv_v[0, t])
            kv0_tiles[t] = kvt

        _kv0(0)
        wq_sb = singles.tile([128, DC, D], BF16)
        nc.scalar.dma_start(out=wq_sb, in_=wq_v)
        _kv0(1)
        qT00 = qtp.tile([128, DC, D], BF16, tag="qT", name="qT00")
        nc.sync.dma_start(out=qT00, in_=q_v[0, 0])
        qT0_pre[0] = qT00
        wk_sb = singles.tile([128, DC, D], BF16)
        nc.scalar.dma_start(out=wk_sb, in_=wk_v)
        wv_sb = singles.tile([128, DC, D], BF16)
        nc.scalar.dma_start(out=wv_sb, in_=wv_v)
        _kv0(2)
        _kv0(3)
        qT01 = qtp.tile([128, DC, D], BF16, tag="qT", name="qT01")
        nc.sync.dma_start(out=qT01, in_=q_v[0, 1])
        qT0_pre[1] = qT01
        wo_sb = singles.tile([128, DC, D], BF16)
        nc.scalar.dma_start(out=wo_sb, in_=wo_v)
        ident_sb = singles.tile([128, 128], F32R)
        nc.scalar.dma_start(out=ident_sb, in_=ident[:, :])
        ones8_sb = singles.tile([128, 4, 8], BF16)
        nc.scalar.dma_start(out=ones8_sb, in_=ones8[:, :, :])
        expand8_sb = singles.tile([8, 4, 128], F32R)
        nc.scalar.dma_start(out=expand8_sb, in_=expand8[:, :, :])
        _kv0(4)
        _kv0(5)

        def make_prep(b):
            """Returns (issue_dmas, chunk_steps, tail_fn, state).
            issue_dmas() starts the qT + kv prefetch for this batch (qT
            first so the next hq never stalls behind 4MiB of kv).  Each
            chunk step then only emits pool matmuls on the pre-fetched
            tile; steps are interleaved into other units' emission so the
            DMA-paced pool never stalls the in-order PE queue."""
            st = {}
            pacc = accp.tile([MEM, D], F32, tag="acc")
            kv_tiles = {}

            def issue_dmas():
                for ic in range(ICN if b > 0 else 0):
                    qTt = qtp.tile([128, DC, D], BF16, tag="qT")
                    nc.sync.dma_start(out=qTt, in_=q_v[b, ic])
                    st[f"qT{ic}"] = qTt
                for t in range(KT):
                    if b == 0 and t in kv0_tiles:
                        kv_tiles[t] = kv0_tiles[t]
                    else:
                        kvt = kvp.tile([128, 4, D], BF16, tag="kv")
                        nc.sync.dma_start(out=kvt, in_=kv_v[b, t])
                        kv_tiles[t] = kvt

            def chunk_step(t):
                def f():
                    kvt = kv_tiles[t]
                    for c in range(4):
                        sc = 4 * t + c
                        nc.tensor.matmul(
                            pacc,
                            lhsT=poolD_sb[:, 62 - 2 * sc : 126 - 2 * sc],
                            rhs=kvt[:, c, :],
                            start=(sc == 0),
                            stop=(sc == 31),
                        )
                return f

            def tail():
                mem_sb = memp.tile([MEM, D], F32R, tag="mem")
                nc.scalar.copy(out=mem_sb, in_=pacc)
                trt = mmp.tile([128, 4, MEM], F32R, tag="mm")
                for c in range(4):
                    nc.tensor.transpose(
                        trt[:, c, :],
                        mem_sb[:, 128 * c : 128 * (c + 1)],
                        ident_sb[0:MEM, 0:MEM],
                    )
                memT_sb = memp.tile([128, 4, MEM], BF16, tag="memT")
                nc.scalar.copy(out=memT_sb, in_=trt)
                # hk, block-diagonal per head pair, with the 1/8 softmax
                # scale folded in
                hkbd_sb = memp.tile([128, 4, 128], BF16, tag="hkbd")
                nc.scalar.mul(out=hkbd_sb, in_=wk_sb[:, 0, :], mul=0.0)
                for oc in range(4):
                    hk_ps = mmp.tile([128, MEM], F32, tag="mm")
                    for dc in range(DC):
                        nc.tensor.matmul(
                            hk_ps,
                            lhsT=wk_sb[:, dc, 128 * oc : 128 * (oc + 1)],
                            rhs=memT_sb[:, dc, :],
                            start=(dc == 0),
                            stop=(dc == DC - 1),
                        )
                    nc.scalar.mul(
                        out=hkbd_sb[0:64, oc, 0:64], in_=hk_ps[0:64, :], mul=0.125
                    )
                    nc.scalar.mul(
                        out=hkbd_sb[64:128, oc, 64:128],
                        in_=hk_ps[64:128, :],
                        mul=0.125,
                    )
                # hvT (transposed V heads), then block-diagonal hvbdT, then
                # the fused per-head V*Wo product hvo.
                hvbdT_sb = memp.tile([128, 4, 128], BF16, tag="hvbdT")
                nc.scalar.mul(out=hvbdT_sb, in_=wv_sb[:, 0, :], mul=0.0)
                for oc in range(4):
                    hvT_ps = mmp.tile([128, MEM], F32, tag="mm")
                    for dc in range(DC):
                        nc.tensor.matmul(
                            hvT_ps,
                            lhsT=wv_sb[:, dc, 128 * oc : 128 * (oc + 1)],
                            rhs=memT_sb[:, dc, :],
                            start=(dc == 0),
                            stop=(dc == DC - 1),
                        )
                    nc.scalar.copy(out=hvbdT_sb[0:64, oc, 0:64], in_=hvT_ps[0:64, :])
                    nc.scalar.copy(
                        out=hvbdT_sb[64:128, oc, 64:128], in_=hvT_ps[64:128, :]
                    )
                hvo_sb = memp.tile([128, 4, D], BF16, tag="hvo")
                for p2 in range(4):
                    hvo_ps = mmp.tile([128, D], F32, tag="mm")
                    nc.tensor.matmul(
                        hvo_ps,
                        lhsT=hvbdT_sb[:, p2, :],
                        rhs=wo_sb[:, p2, :],
                        start=True,
                        stop=True,
                    )
                    nc.scalar.copy(out=hvo_sb[:, p2, :], in_=hvo_ps)
                st["hkbd"] = hkbd_sb
                st["hvo"] = hvo_sb

            return issue_dmas, [chunk_step(t) for t in range(KT)], tail, st

        def emit_hq(b, ic, qT_pre=None, filler=None):
            """qT load + the hq projection; optionally interleaves pending
            pool-chunk steps between oc groups (used for the batch-0 ramp,
            where the PE would otherwise idle while kv streams in)."""
            if qT_pre is not None:
                qT_sb = qT_pre
            else:
                qT_sb = qtp.tile([128, DC, D], BF16, tag="qT")
                nc.sync.dma_start(out=qT_sb, in_=q_v[b, ic])
            hqT_sb = hqp.tile([128, DC, D], BF16, tag="hqT")
            for oc in range(DC):
                if filler:
                    filler.pop(0)()
                hq_ps = mmp.tile([128, D], F32, tag="mm")
                for dc in range(DC):
                    nc.tensor.matmul(
                        hq_ps,
                        lhsT=wq_sb[:, dc, 128 * oc : 128 * (oc + 1)],
                        rhs=qT_sb[:, dc, :],
                        start=(dc == 0),
                        stop=(dc == DC - 1),
                    )
                nc.scalar.copy(out=hqT_sb[:, oc, :], in_=hq_ps)
            return hqT_sb

        def emit_A(bst, b, ic, hqT_sb):
            """scores, exp, denominators, reciprocal."""
            st = {"b": b, "ic": ic, "bst": bst}
            den_ps = denp.tile([8, D], F32, tag="den")
            e_tiles = []
            for p2 in range(4):
                sc_ps = mmp.tile([128, D], F32, tag="mm")
                nc.tensor.matmul(
                    sc_ps,
                    lhsT=bst["hkbd"][:, p2, :],
                    rhs=hqT_sb[:, p2, :],
                    start=True,
                    stop=True,
                )
                e_sb = ep.tile([128, D], BF16, tag="e")
                nc.scalar.activation(out=e_sb, in_=sc_ps, func=EXPF)
                e_tiles.append(e_sb)
                nc.tensor.matmul(
                    den_ps,
                    lhsT=ones8_sb[:, p2, :],
                    rhs=e_sb,
                    start=(p2 == 0),
                    stop=(p2 == 3),
                )
            rden_sb = rdp.tile([8, D], F32R, tag="rden")
            with nc.allow_low_precision(reason="f32r reciprocal feeds f32r matmul"):
                nc.vector.reciprocal(out=rden_sb, in_=den_ps)
            st["e"] = e_tiles
            st["rden"] = rden_sb
            return st

        def emit_B(st, filler, split_store=False):
            """broadcast denominators, normalize, fused V*Wo projection,
            store for unit st.  After each head pair, emit one pending
            pool-chunk of the next batch (keeps the PE queue fed while this
            unit's DVE/ACT run)."""
            b, ic, bst = st["b"], st["ic"], st["bst"]
            attn_tiles = []
            for p2 in range(4):
                if filler:
                    filler.pop(0)()
                bc_ps = mmp.tile([128, D], F32, tag="mm")
                nc.tensor.matmul(
                    bc_ps,
                    lhsT=expand8_sb[:, p2, :],
                    rhs=st["rden"],
                    start=True,
                    stop=True,
                )
                attn_sb = attnp.tile([128, D], BF16, tag="attn")
                nc.vector.tensor_mul(attn_sb, st["e"][p2], bc_ps)
                attn_tiles.append(attn_sb)
            out_sb = outp.tile([128, IT, D], BF16, tag="o")
            for it in range(IT):
                o_ps = mmp.tile([128, D], F32, tag="mm")
                for p2 in range(4):
                    nc.tensor.matmul(
                        o_ps,
                        lhsT=attn_tiles[p2][:, 128 * it : 128 * (it + 1)],
                        rhs=bst["hvo"][:, p2, :],
                        start=(p2 == 0),
                        stop=(p2 == 3),
                    )
                nc.vector.tensor_copy(out=out_sb[:, it, :], in_=o_ps)
                if split_store:
                    nc.scalar.dma_start(
                        out=out_v[b, ic][:, it, :], in_=out_sb[:, it, :]
                    )
            if not split_store:
                # store on the second HWDGE ring (ACT) so loads/stores overlap
                nc.scalar.dma_start(out=out_v[b, ic], in_=out_sb)

        # software pipeline: A(k+1) is emitted before B(k) so the PE has
        # matmul work while unit k's reciprocal runs; the next batch's
        # DMA-paced pool chunks are woven into B phases.  Batch 0's ramp
        # interleaves its hq projections with the kv0-paced pool chunks so
        # the PE has non-DMA-gated work from the first microsecond.
        issue0, steps0, tail0, st0 = make_prep(0)
        issue0()
        hq00 = emit_hq(0, 0, qT_pre=qT0_pre[0], filler=steps0)
        hq01 = emit_hq(0, 1, qT_pre=qT0_pre[1], filler=steps0)
        for f in steps0:
            f()
        tail0()
        batch_state = {0: st0}
        hq_pre = {(0, 0): hq00, (0, 1): hq01}
        for b in range(B):
            h0 = hq_pre.pop((b, 0), None)
            if h0 is None:
                h0 = emit_hq(b, 0, qT_pre=batch_state[b].get("qT0"))
            stA0 = emit_A(batch_state[b], b, 0, h0)
            h1 = hq_pre.pop((b, 1), None)
            if h1 is None:
                h1 = emit_hq(b, 1, qT_pre=batch_state[b].get("qT1"))
            stA1 = emit_A(batch_state[b], b, 1, h1)
            if b + 1 < B:
                next_issue, next_steps, next_tail, next_st = make_prep(b + 1)
                # start the qT + kv prefetch now: the B phases below only
                # hold ~8.5us of PE work while 5MiB of batch b+1 input
                # needs ~14us of DMA -- issuing here hides the difference.
                next_issue()
            else:
                next_steps, next_tail, next_st = [], None, None
            emit_B(stA0, next_steps)
            emit_B(stA1, next_steps, split_store=(b == B - 1))
            for f in next_steps:
                f()
            if next_tail is not None:
                # emit the next batch's first hq before its prep tail: the
                # hq matmuls depend only on qT/wq, so the PE chews on them
                # while the tail's ACT-chained mem/memT/hkbd copies and
                # small matmuls trickle through (instead of idling).
                hq_pre[(b + 1, 0)] = emit_hq(
                    b + 1, 0, qT_pre=next_st.get("qT0")
                )
                next_tail()
                batch_state[b + 1] = next_st
                hq_pre[(b + 1, 1)] = emit_hq(
                    b + 1, 1, qT_pre=next_st.get("qT1")
                )
    return nc


_NC = None


def _get_nc() -> bass.Bass:
    global _NC
    if _NC is None:
        _NC = _build_nc()
    return _NC


def _consts() -> dict:
    bf = ml_dtypes.bfloat16
    poolD = np.zeros((128, 126), np.float32)
    poolD[0:64, 62] = 1.0 / 64.0
    poolD[64:128, 63] = 1.0 / 64.0
    ident = np.eye(128, dtype=np.float32)
    # den matmul lhsT (per pair p2): accumulate into one [8, D] tile; row
    # 2*p2 sums even-head exp rows (partitions 0-63), row 2*p2+1 odd
    # (64-127).  All four pair matmuls accumulate into the same tile, so
    # every row ends up with exactly one head's denominator.
    ones8 = np.zeros((128, 4, 8), np.float32)
    for p2 in range(4):
        ones8[0:64, p2, 2 * p2] = 1.0
        ones8[64:128, p2, 2 * p2 + 1] = 1.0
    # broadcast matmul lhsT, one [8, 128] slice per head pair
    expand8 = np.zeros((8, 4, 128), np.float32)
    for p2 in range(4):
        expand8[2 * p2 + 0, p2, 0:64] = 1.0
        expand8[2 * p2 + 1, p2, 64:128] = 1.0
    return dict(
        poolD=poolD.astype(bf),
        ident=ident,
        ones8=ones8.astype(bf),
        expand8=expand8,
    )


def run(inputs: dict, trace: bool = False):
    """Run on 8 cores; returns (full_output, BassKernelResults)."""
    bf = ml_dtypes.bfloat16
    query = np.asarray(inputs["query"], np.float32)
    queryT = np.ascontiguousarray(query.transpose(0, 2, 1)).astype(bf)
    keyvalue = np.asarray(inputs["keyvalue"], np.float32).astype(bf)
    w = {
        "wqT": np.ascontiguousarray(np.asarray(inputs["Wq"], np.float32).T).astype(bf),
        "wkT": np.ascontiguousarray(np.asarray(inputs["Wk"], np.float32).T).astype(bf),
        "wvT": np.ascontiguousarray(np.asarray(inputs["Wv"], np.float32).T).astype(bf),
        "woT": np.ascontiguousarray(np.asarray(inputs["Wo"], np.float32).T).astype(bf),
    }
    consts = _consts()
    nb = query.shape[0]
    per = nb // NCORES
    assert per == B, f"expected {NCORES * B} batches, got {nb}"

    in_maps = []
    for k in range(NCORES):
        m = {
            "queryT": np.ascontiguousarray(queryT[k * per : (k + 1) * per]),
            "keyvalue": np.ascontiguousarray(keyvalue[k * per : (k + 1) * per]),
        }
        m.update(w)
        m.update(consts)
        in_maps.append(m)

    res = run_bass_kernel_spmd(
        _get_nc(), in_maps, core_ids=list(range(NCORES)), trace=trace
    )
    outs = [r["out"].astype(np.float32) for r in res.results]
    return np.concatenate(outs, axis=0), res


def kernel(**inputs) -> np.ndarray:
    out, _ = run(inputs, trace=False)
    return out
